# revision 17
# baseline (speedup 1.0000x reference)
"""Distributed Trainium2 kernel for GPT-2 style multi-head causal attention.

reference:
    qkv = x @ w_attn + b_attn            # [B,S,3*NX]
    q,k,v split; 16 heads, DH=64; causal softmax(q k^T / sqrt(DH)) v
    out = a @ w_proj + b_proj            # [B,S,NX]

Sharding over 8 NeuronCores: core c -> (batch b=c//2, head-group g=c%2).
Each core computes qkv for its batch and its 8 heads (Megatron column-parallel
c_attn), flash attention for those 8 heads fully in SBUF, a per-head-pair
2-core AllGather of the attention outputs (overlapped under later pairs'
compute), and a column-parallel c_proj (each core produces 512 of the 1024
output features for all 2048 tokens of its batch). Host concatenates.

Compute in bf16 on the TensorEngine with fp32 PSUM accumulation; softmax is
the "unsafe" variant (no row-max subtraction) which is exact here: score
magnitudes are O(1) and masked lanes are multiplied by an exact 0/1 mask
after exp. Softmax denominators come from a ones-column appended to V.
"""

import sys

if "/opt/trn_rl_repo" not in sys.path:
    sys.path.insert(0, "/opt/trn_rl_repo")

import numpy as np
import ml_dtypes

import concourse.bass as bass
import concourse.mybir as mybir
import concourse.tile as tile
from concourse import bacc
from concourse.bass_utils import run_bass_kernel_spmd

BF16 = ml_dtypes.bfloat16

B, S, NX, H = 4, 2048, 1024, 16
DH = NX // H  # 64
N_CORES = 8
HPC = 8          # heads per core
FQK = HPC * DH   # 512 q (or k) features per core
GQ = S // 512    # 4 q-tiles of 512
TT16 = S // 128  # 16 token chunks of 128

f32 = mybir.dt.float32
bf16 = mybir.dt.bfloat16
f8 = mybir.dt.float8e4
F8NP = ml_dtypes.float8_e4m3

_BUILD_CACHE: dict = {}


def build_nc(debug_taps: bool = False, reps: int = 1, sim_single: bool = False,
             phase_limit: int = 3, bc_mode: str = "gpsimd", no_cc: bool = False,
             g1a_fp8: bool = False):
    """Build + compile the SPMD Bass graph (identical on all 8 cores).

    reps>1 replicates the whole body (for slope-based timing: the axon
    dispatch overhead is large, so per-exec time = slope of wall vs reps).
    sim_single builds a 1-core variant with the collective replaced by
    equivalent local DMAs, for TimelineSim cost-model profiling.
    phase_limit: 1=GEMM1 only, 2=+flash, 3=full (for phase attribution).
    """
    key = ("nc", debug_taps, reps, sim_single, phase_limit, bc_mode, no_cc, g1a_fp8)
    if key in _BUILD_CACHE:
        return _BUILD_CACHE[key]

    ndev = 1 if sim_single else N_CORES
    nc = bacc.Bacc("TRN2", target_bir_lowering=False, debug=False, num_devices=ndev)

    xT = nc.dram_tensor("xT", [NX, S], bf16, kind="ExternalInput")
    if g1a_fp8:
        xT8 = nc.dram_tensor("xT8", [NX, S], f8, kind="ExternalInput")
        wqk8 = nc.dram_tensor("wqk8", [NX, 2 * FQK], f8, kind="ExternalInput")
    else:
        wqk = nc.dram_tensor("wqk", [NX, 2 * FQK], bf16, kind="ExternalInput")
    wv = nc.dram_tensor("wv", [NX, FQK], bf16, kind="ExternalInput")
    bqk = nc.dram_tensor("bqk", [128, 8], f32, kind="ExternalInput")
    bvb = nc.dram_tensor("bvb", [128, FQK], f32, kind="ExternalInput")
    wpj = nc.dram_tensor("wpj", [NX, FQK], bf16, kind="ExternalInput")
    bpj = nc.dram_tensor("bpj", [128, FQK], f32, kind="ExternalInput")
    msk = nc.dram_tensor("msk", [128, 4, 512], bf16, kind="ExternalInput")
    out = nc.dram_tensor("out", [S, FQK], f32, kind="ExternalOutput")
    if debug_taps:
        qkT_tap = nc.dram_tensor("qkT_tap", [128, 8, S], f32, kind="ExternalOutput")
        von_tap = nc.dram_tensor("von_tap", [128, TT16, HPC, 65], f32, kind="ExternalOutput")
        aT_tap = nc.dram_tensor("aT_tap", [128, 4, S], f32, kind="ExternalOutput")

    groups = [[0, 1], [2, 3], [4, 5], [6, 7]]

    with tile.TileContext(nc) as tc:
      for _rep in range(reps):
        # internal DRAM for the per-pair collectives
        ag_ins = [nc.dram_tensor(f"ag_in{_rep}_{i}", [128, S], bf16) for i in range(4)]
        ag_outs = [nc.dram_tensor(f"ag_out{_rep}_{i}", [256, S], bf16) for i in range(4)]
        with (
            tc.tile_pool(name="persist", bufs=1) as pp,
            tc.tile_pool(name="ptmp", bufs=4) as ptmp,
            tc.tile_pool(name="ptmp2", bufs=2) as ptmp2,
            tc.tile_pool(name="oevict", bufs=3) as oev,
            tc.tile_pool(name="ps_s", bufs=2, space="PSUM") as ps_s,
            tc.tile_pool(name="ps_a", bufs=2, space="PSUM") as ps_a,
            tc.tile_pool(name="ps_r", bufs=1, space="PSUM") as ps_r,
        ):
            # ---- persistent SBUF tensors
            qkT = pp.tile([128, 8, S], bf16)       # fc 0-3: q, 4-7: k (2 heads/chunk)
            von = pp.tile([128, TT16, HPC, 65], bf16)  # v natural + ones column
            aT = pp.tile([128, 4, S], bf16)        # per-head-pair attn out (f' x t)
            gath = pp.tile([128, 8, S], bf16)      # AllGathered aT (both groups)
            msk_sb = pp.tile([128, 4, 512], bf16)
            bqk_sb = pp.tile([128, 8], f32)
            bvb_sb = pp.tile([128, FQK], f32)
            bpj_sb = pp.tile([128, FQK], f32)
            wpj_sb = pp.tile([128, 8, FQK], bf16)
            ones1 = pp.tile([1, 64], f32)
            zb = pp.tile([128, 1], f32)

            nc.sync.dma_start(msk_sb[:], msk[:])
            nc.sync.dma_start(bqk_sb[:], bqk[:])
            nc.sync.dma_start(bvb_sb[:], bvb[:])
            nc.sync.dma_start(bpj_sb[:], bpj[:])
            nc.sync.dma_start(wpj_sb[:], wpj[:].rearrange("(c p) f -> p c f", p=128))
            nc.vector.memset(ones1[:], 1.0)
            nc.vector.memset(zb[:], 0.0)
            # ones column of von (softmax denominator trick)
            nc.vector.memset(von[:, :, :, 64:65], 1.0)

            with (
                tc.tile_pool(name="g1", bufs=1) as g1p,
                tc.tile_pool(name="ps_g1", bufs=(2 if bc_mode == "gpsimd" else 1),
                             space="PSUM") as ps_g1,
            ):
                xT_sb = g1p.tile([128, 8, S], bf16)
                wv_sb = g1p.tile([128, 8, FQK], bf16)
                # split input DMAs; GEMM1b needs wv + xT quarter 0 first
                nc.sync.dma_start(wv_sb[:], wv[:].rearrange("(c p) f -> p c f", p=128))
                for q in range(4):
                    sl = slice(q * (S // 4), (q + 1) * (S // 4))
                    nc.sync.dma_start(
                        xT_sb[:, :, sl],
                        xT[:, sl].rearrange("(c p) t -> p c t", p=128),
                    )
                if g1a_fp8:
                    xT8_sb = g1p.tile([128, 8, S], f8)
                    wqk8_sb = g1p.tile([128, 8, 2 * FQK], f8)
                    nc.sync.dma_start(
                        xT8_sb[:], xT8[:].rearrange("(c p) t -> p c t", p=128)
                    )
                    nc.sync.dma_start(
                        wqk8_sb[:], wqk8[:].rearrange("(c p) f -> p c f", p=128)
                    )
                else:
                    wqk_sb = g1p.tile([128, 8, 2 * FQK], bf16)
                    nc.sync.dma_start(
                        wqk_sb[:], wqk[:].rearrange("(c p) f -> p c f", p=128)
                    )

                def gemm1b_chunk(tt):
                    # v natural layout [t, h, d] (+ones col kept intact)
                    ps = ps_g1.tile([128, 512], f32, tag="g1")
                    for c in range(8):
                        nc.tensor.matmul(
                            ps[:],
                            xT_sb[:, c, tt * 128 : (tt + 1) * 128],
                            wv_sb[:, c, :],
                            start=(c == 0),
                            stop=(c == 7),
                        )
                    nc.vector.tensor_tensor(
                        von[:, tt, :, 0:64],
                        ps[:].rearrange("p (h d) -> p h d", d=64),
                        bvb_sb[:].rearrange("p (h d) -> p h d", d=64),
                        mybir.AluOpType.add,
                    )

                def gemm1a_chunk(fc):
                    # q,k transposed layout [f, t]
                    for tt in range(GQ):
                        ps = ps_g1.tile([128, 512], f32, tag="g1")
                        if g1a_fp8:
                            # DoubleRow: 2 fp8 K-planes per PE cell, K=256/mm
                            for c2 in range(4):
                                nc.tensor.matmul(
                                    ps[:],
                                    wqk8_sb[:, 2 * c2 : 2 * c2 + 2,
                                            fc * 128 : (fc + 1) * 128],
                                    xT8_sb[:, 2 * c2 : 2 * c2 + 2,
                                           tt * 512 : (tt + 1) * 512],
                                    start=(c2 == 0),
                                    stop=(c2 == 3),
                                    perf_mode=mybir.MatmulPerfMode.DoubleRow,
                                )
                        else:
                            for c in range(8):
                                nc.tensor.matmul(
                                    ps[:],
                                    wqk_sb[:, c, fc * 128 : (fc + 1) * 128],
                                    xT_sb[:, c, tt * 512 : (tt + 1) * 512],
                                    start=(c == 0),
                                    stop=(c == 7),
                                )
                        nc.vector.tensor_scalar(
                            qkT[:, fc, tt * 512 : (tt + 1) * 512],
                            ps[:],
                            bqk_sb[:, fc : fc + 1],
                            None,
                            mybir.AluOpType.add,
                        )

                def flash_pair_qt(i, qt):
                    # heads 2i (partitions 0-63) and 2i+1 (partitions 64-127)
                    nkc = 4 * (qt + 1)
                    a_ps = [
                        ps_a.tile([65, 512], f32, tag="aT", name=f"aps{i}_{qt}_{h}")
                        for h in range(2)
                    ]

                    def scores(kc):
                        # both heads' QK^T into one 2-bank psum tile, one
                        # fused exp, one fused causal mask
                        j = kc - 4 * qt  # >=0 -> diagonal-overlap chunk
                        sT = ps_s.tile(
                            [128, 2, 512], f32, tag="sT", name=f"sT{i}_{qt}_{kc}"
                        )
                        for h in range(2):
                            p0 = 64 * h
                            nc.tensor.matmul(
                                sT[:, h, :],
                                qkT[p0 : p0 + 64, 4 + i, kc * 128 : (kc + 1) * 128],
                                qkT[p0 : p0 + 64, i, qt * 512 : (qt + 1) * 512],
                                start=True,
                                stop=True,
                            )
                        pT = ptmp.tile(
                            [128, 2, 512], bf16, tag="pT", name=f"pT{i}_{qt}_{kc}"
                        )
                        nc.scalar.activation(
                            pT[:],
                            sT[:],
                            mybir.ActivationFunctionType.Exp,
                            bias=zb[:],
                            scale=0.125,
                        )
                        if j >= 0:
                            nc.vector.tensor_tensor(
                                pT[:],
                                pT[:],
                                msk_sb[:, j : j + 1, :].to_broadcast((128, 2, 512)),
                                mybir.AluOpType.mult,
                            )
                        return pT

                    def pv(kc, pT):
                        for h in range(2):
                            nc.tensor.matmul(
                                a_ps[h][:],
                                von[:, kc, 2 * i + h, :],
                                pT[:, h, :],
                                start=(kc == 0),
                                stop=(kc == nkc - 1),
                            )

                    # software pipeline: PV lags scores by one chunk so the
                    # PE never stalls on the ACT exp of the current chunk
                    prev = scores(0)
                    for kc in range(1, nkc):
                        cur = scores(kc)
                        pv(kc - 1, prev)
                        prev = cur
                    pv(nkc - 1, prev)

                    # normalize by the ones-row denominator and store bf16
                    for h in range(2):
                        rec = ptmp2.tile([1, 512], f32, tag="rec")
                        nc.vector.reciprocal(rec[:], a_ps[h][64:65, :])
                        rb_sb = ptmp2.tile([64, 512], f32, tag="rbs")
                        if bc_mode == "gpsimd":
                            nc.gpsimd.partition_broadcast(rb_sb[:], rec[:])
                        else:
                            rb = ps_r.tile([64, 512], f32, tag="rb")
                            nc.tensor.matmul(rb[:], ones1[:], rec[:],
                                             start=True, stop=True)
                            nc.vector.tensor_copy(rb_sb[:], rb[:])
                        nc.vector.tensor_tensor(
                            aT[64 * h : 64 * h + 64, i, qt * 512 : (qt + 1) * 512],
                            a_ps[h][0:64, :],
                            rb_sb[:],
                            mybir.AluOpType.mult,
                        )

                def allgather_pair(i):
                    # ship pair i's attention output while later pairs compute
                    nc.sync.dma_start(ag_ins[i][:], aT[:, i, :])
                    if sim_single or no_cc:
                        nc.sync.dma_start(ag_outs[i][0:128, :], ag_ins[i][:])
                        nc.sync.dma_start(ag_outs[i][128:256, :], ag_ins[i][:])
                    else:
                        nc.gpsimd.collective_compute(
                            "AllGather",
                            mybir.AluOpType.bypass,
                            replica_groups=groups,
                            ins=[ag_ins[i][:].opt()],
                            outs=[ag_outs[i][:].opt()],
                        )
                    # prefetch into SBUF for GEMM2 while flash continues
                    for g in range(2):
                        nc.sync.dma_start(
                            gath[:, g * 4 + i, :],
                            ag_outs[i][g * 128 : (g + 1) * 128, :],
                        )

                def gemm2_pass1():
                    # first half of c_proj contraction (pairs 0,1 of both
                    # groups, already AllGathered) hidden under flash pairs
                    # 2-3; writes fp32 partials to `out`, pass2 accumulates.
                    for tt in range(TT16):
                        ps = ps_g1.tile([128, 512], f32, tag="g1")
                        for ci, c in enumerate([0, 4, 1, 5]):
                            nc.tensor.matmul(
                                ps[:],
                                gath[:, c, tt * 128 : (tt + 1) * 128],
                                wpj_sb[:, c, :],
                                start=(ci == 0),
                                stop=(ci == 3),
                            )
                        og = oev.tile([128, 512], f32, tag="og")
                        nc.vector.tensor_tensor(
                            og[:], ps[:], bpj_sb[:], mybir.AluOpType.add
                        )
                        nc.sync.dma_start(out[tt * 128 : (tt + 1) * 128, :], og[:])

                # GEMM1a for pair 0 first; GEMM1b chunks just-in-time under
                # flash pair 0; later pairs' GEMM1a between flash pairs.
                gemm1a_chunk(0)
                gemm1a_chunk(4)
                for i in range(4):
                    if phase_limit >= 2:
                        for qt in range(GQ):
                            if i == 0:
                                for tt in range(4 * qt, 4 * qt + 4):
                                    gemm1b_chunk(tt)
                            flash_pair_qt(i, qt)
                    elif i == 0:
                        for tt in range(TT16):
                            gemm1b_chunk(tt)
                    if i < 3:
                        gemm1a_chunk(i + 1)
                        gemm1a_chunk(5 + i)
                    if phase_limit >= 3:
                        allgather_pair(i)
                        if i == 2:
                            gemm2_pass1()

            if phase_limit < 3:
                continue
            # ---- GEMM2 pass2: remaining contraction chunks (pairs 2,3),
            # accumulated into `out` by the DMA engine's inline adder.
            with tc.tile_pool(name="ps_g2", bufs=2, space="PSUM") as ps_g2:
                for tt in range(TT16):
                    ps = ps_g2.tile([128, 512], f32, tag="g2")
                    for ci, c in enumerate([2, 6, 3, 7]):
                        nc.tensor.matmul(
                            ps[:],
                            gath[:, c, tt * 128 : (tt + 1) * 128],
                            wpj_sb[:, c, :],
                            start=(ci == 0),
                            stop=(ci == 3),
                        )
                    og = oev.tile([128, 512], f32, tag="og")
                    nc.vector.tensor_copy(og[:], ps[:])
                    nc.gpsimd.dma_start(
                        out[tt * 128 : (tt + 1) * 128, :], og[:],
                        accum_op=mybir.AluOpType.add,
                    )

            if debug_taps:
                tq = oev.tile([128, 8, S], f32, tag="tapq")
                nc.vector.tensor_copy(tq[:], qkT[:])
                nc.sync.dma_start(qkT_tap[:], tq[:])
                tv = oev.tile([128, TT16, HPC, 65], f32, tag="tapv")
                nc.vector.tensor_copy(tv[:], von[:])
                nc.sync.dma_start(von_tap[:], tv[:])
                ta = oev.tile([128, 4, S], f32, tag="tapa")
                nc.vector.tensor_copy(ta[:], aT[:])
                nc.sync.dma_start(aT_tap[:], ta[:])

    nc.compile()
    _BUILD_CACHE[key] = nc
    return nc


def make_in_maps(x, w_attn, b_attn, w_proj, b_proj):
    """Shard the full inputs into 8 per-core input maps."""
    x = np.asarray(x, dtype=np.float32)
    w_attn = np.asarray(w_attn, dtype=np.float32)
    b_attn = np.asarray(b_attn, dtype=np.float32)
    w_proj = np.asarray(w_proj, dtype=np.float32)
    b_proj = np.asarray(b_proj, dtype=np.float32)

    kp = np.arange(128)[:, None, None]
    jj = np.arange(4)[None, :, None]
    qf = np.arange(512)[None, None, :]
    mask = (kp + 128 * jj <= qf).astype(BF16)

    in_maps = []
    for c in range(N_CORES):
        b, g = c // 2, c % 2
        sl = slice(g * FQK, (g + 1) * FQK)
        wq = w_attn[:, 0 * NX :][:, sl]
        wk = w_attn[:, 1 * NX :][:, sl]
        wv_ = w_attn[:, 2 * NX :][:, sl]
        bq = b_attn[0 * NX :][sl]
        bk = b_attn[1 * NX :][sl]
        bv_ = b_attn[2 * NX :][sl]
        xt = np.ascontiguousarray(x[b].T)
        wqk_full = np.ascontiguousarray(np.concatenate([wq, wk], axis=1))
        in_maps.append(
            {
                "xT": xt.astype(BF16),
                "xT8": xt.astype(F8NP),
                "wqk": wqk_full.astype(BF16),
                "wqk8": wqk_full.astype(F8NP),
                "wv": np.ascontiguousarray(wv_).astype(BF16),
                "bqk": np.ascontiguousarray(
                    np.concatenate([bq, bk]).reshape(8, 128).T
                ).astype(np.float32),
                "bvb": np.ascontiguousarray(
                    np.broadcast_to(bv_[None, :], (128, FQK))
                ).astype(np.float32),
                "wpj": np.ascontiguousarray(w_proj[:, sl]).astype(BF16),
                "bpj": np.ascontiguousarray(
                    np.broadcast_to(b_proj[None, sl], (128, FQK))
                ).astype(np.float32),
                "msk": mask,
            }
        )
    return in_maps


def assemble_out(results):
    out = np.empty((B, S, NX), dtype=np.float32)
    for c in range(N_CORES):
        b, g = c // 2, c % 2
        out[b, :, g * FQK : (g + 1) * FQK] = results[c]["out"]
    return out


def kernel(x, w_attn, b_attn, w_proj, b_proj):
    nc = build_nc()
    in_maps = make_in_maps(x, w_attn, b_attn, w_proj, b_proj)
    res = run_bass_kernel_spmd(nc, in_maps, core_ids=list(range(N_CORES)))
    return assemble_out(res.results)


# revision 18
# speedup vs baseline: 1.0098x; 1.0098x over previous
"""Distributed Trainium2 kernel for GPT-2 style multi-head causal attention.

reference:
    qkv = x @ w_attn + b_attn            # [B,S,3*NX]
    q,k,v split; 16 heads, DH=64; causal softmax(q k^T / sqrt(DH)) v
    out = a @ w_proj + b_proj            # [B,S,NX]

Sharding over 8 NeuronCores: core c -> (batch b=c//2, head-group g=c%2).
Each core computes qkv for its batch and its 8 heads (Megatron column-parallel
c_attn), flash attention for those 8 heads fully in SBUF, a per-head-pair
2-core AllGather of the attention outputs (overlapped under later pairs'
compute), and a column-parallel c_proj (each core produces 512 of the 1024
output features for all 2048 tokens of its batch). Host concatenates.

Compute in bf16 on the TensorEngine with fp32 PSUM accumulation; softmax is
the "unsafe" variant (no row-max subtraction) which is exact here: score
magnitudes are O(1) and masked lanes are multiplied by an exact 0/1 mask
after exp. Softmax denominators come from a ones-column appended to V.
"""

import sys

if "/opt/trn_rl_repo" not in sys.path:
    sys.path.insert(0, "/opt/trn_rl_repo")

import numpy as np
import ml_dtypes

import concourse.bass as bass
import concourse.mybir as mybir
import concourse.tile as tile
from concourse import bacc
from concourse.bass_utils import run_bass_kernel_spmd

BF16 = ml_dtypes.bfloat16

B, S, NX, H = 4, 2048, 1024, 16
DH = NX // H  # 64
N_CORES = 8
HPC = 8          # heads per core
FQK = HPC * DH   # 512 q (or k) features per core
GQ = S // 512    # 4 q-tiles of 512
TT16 = S // 128  # 16 token chunks of 128

f32 = mybir.dt.float32
bf16 = mybir.dt.bfloat16
f8 = mybir.dt.float8e4
F8NP = ml_dtypes.float8_e4m3

_BUILD_CACHE: dict = {}


def build_nc(debug_taps: bool = False, reps: int = 1, sim_single: bool = False,
             phase_limit: int = 3, bc_mode: str = "gpsimd", no_cc: bool = False,
             g1a_fp8: bool = False):
    """Build + compile the SPMD Bass graph (identical on all 8 cores).

    reps>1 replicates the whole body (for slope-based timing: the axon
    dispatch overhead is large, so per-exec time = slope of wall vs reps).
    sim_single builds a 1-core variant with the collective replaced by
    equivalent local DMAs, for TimelineSim cost-model profiling.
    phase_limit: 1=GEMM1 only, 2=+flash, 3=full (for phase attribution).
    """
    key = ("nc", debug_taps, reps, sim_single, phase_limit, bc_mode, no_cc, g1a_fp8)
    if key in _BUILD_CACHE:
        return _BUILD_CACHE[key]

    ndev = 1 if sim_single else N_CORES
    nc = bacc.Bacc("TRN2", target_bir_lowering=False, debug=False, num_devices=ndev)

    xT = nc.dram_tensor("xT", [NX, S], bf16, kind="ExternalInput")
    if g1a_fp8:
        xT8 = nc.dram_tensor("xT8", [NX, S], f8, kind="ExternalInput")
        wqk8 = nc.dram_tensor("wqk8", [NX, 2 * FQK], f8, kind="ExternalInput")
    else:
        wqk = nc.dram_tensor("wqk", [NX, 2 * FQK], bf16, kind="ExternalInput")
    wv = nc.dram_tensor("wv", [NX, FQK], bf16, kind="ExternalInput")
    bqk = nc.dram_tensor("bqk", [128, 8], f32, kind="ExternalInput")
    bvb = nc.dram_tensor("bvb", [128, FQK], f32, kind="ExternalInput")
    wpj = nc.dram_tensor("wpj", [NX, FQK], bf16, kind="ExternalInput")
    bpj = nc.dram_tensor("bpj", [128, FQK], f32, kind="ExternalInput")
    msk = nc.dram_tensor("msk", [128, 4, 512], bf16, kind="ExternalInput")
    out = nc.dram_tensor("out", [S, FQK], f32, kind="ExternalOutput")
    if debug_taps:
        qkT_tap = nc.dram_tensor("qkT_tap", [128, 8, S], f32, kind="ExternalOutput")
        von_tap = nc.dram_tensor("von_tap", [128, TT16, HPC, 65], f32, kind="ExternalOutput")
        aT_tap = nc.dram_tensor("aT_tap", [128, 4, S], f32, kind="ExternalOutput")

    groups = [[0, 1], [2, 3], [4, 5], [6, 7]]

    with tile.TileContext(nc) as tc:
      for _rep in range(reps):
        # internal DRAM for the per-pair collectives
        ag_ins = [nc.dram_tensor(f"ag_in{_rep}_{i}", [128, S], bf16) for i in range(4)]
        ag_outs = [nc.dram_tensor(f"ag_out{_rep}_{i}", [256, S], bf16) for i in range(4)]
        with (
            tc.tile_pool(name="persist", bufs=1) as pp,
            tc.tile_pool(name="ptmp", bufs=4) as ptmp,
            tc.tile_pool(name="ptmp2", bufs=2) as ptmp2,
            tc.tile_pool(name="oevict", bufs=3) as oev,
            tc.tile_pool(name="ps_s", bufs=2, space="PSUM") as ps_s,
            tc.tile_pool(name="ps_a", bufs=2, space="PSUM") as ps_a,
            tc.tile_pool(name="ps_r", bufs=1, space="PSUM") as ps_r,
        ):
            # ---- persistent SBUF tensors
            qkT = pp.tile([128, 8, S], bf16)       # fc 0-3: q, 4-7: k (2 heads/chunk)
            von = pp.tile([128, TT16, HPC, 65], bf16)  # v natural + ones column
            aT = pp.tile([128, 4, S], bf16)        # per-head-pair attn out (f' x t)
            gath = pp.tile([128, 8, S], bf16)      # AllGathered aT (both groups)
            msk_sb = pp.tile([128, 4, 512], bf16)
            bqk_sb = pp.tile([128, 8], f32)
            bvb_sb = pp.tile([128, FQK], f32)
            bpj_sb = pp.tile([128, FQK], f32)
            wpj_sb = pp.tile([128, 8, FQK], bf16)
            ones1 = pp.tile([1, 64], f32)
            zb = pp.tile([128, 1], f32)

            nc.sync.dma_start(msk_sb[:], msk[:])
            nc.sync.dma_start(bqk_sb[:], bqk[:])
            nc.sync.dma_start(bvb_sb[:], bvb[:])
            nc.sync.dma_start(bpj_sb[:], bpj[:])
            nc.sync.dma_start(wpj_sb[:], wpj[:].rearrange("(c p) f -> p c f", p=128))
            nc.vector.memset(ones1[:], 1.0)
            nc.vector.memset(zb[:], 0.0)
            # ones column of von (softmax denominator trick)
            nc.vector.memset(von[:, :, :, 64:65], 1.0)

            with (
                tc.tile_pool(name="g1", bufs=1) as g1p,
                tc.tile_pool(name="ps_g1", bufs=(2 if bc_mode == "gpsimd" else 1),
                             space="PSUM") as ps_g1,
            ):
                xT_sb = g1p.tile([128, 8, S], bf16)
                wv_sb = g1p.tile([128, 8, FQK], bf16)
                # split input DMAs; GEMM1b needs wv + xT quarter 0 first
                nc.sync.dma_start(wv_sb[:], wv[:].rearrange("(c p) f -> p c f", p=128))
                for q in range(4):
                    sl = slice(q * (S // 4), (q + 1) * (S // 4))
                    nc.sync.dma_start(
                        xT_sb[:, :, sl],
                        xT[:, sl].rearrange("(c p) t -> p c t", p=128),
                    )
                if g1a_fp8:
                    xT8_sb = g1p.tile([128, 8, S], f8)
                    wqk8_sb = g1p.tile([128, 8, 2 * FQK], f8)
                    nc.sync.dma_start(
                        xT8_sb[:], xT8[:].rearrange("(c p) t -> p c t", p=128)
                    )
                    nc.sync.dma_start(
                        wqk8_sb[:], wqk8[:].rearrange("(c p) f -> p c f", p=128)
                    )
                else:
                    wqk_sb = g1p.tile([128, 8, 2 * FQK], bf16)
                    nc.sync.dma_start(
                        wqk_sb[:], wqk[:].rearrange("(c p) f -> p c f", p=128)
                    )

                def gemm1b_chunk(tt):
                    # v natural layout [t, h, d] (+ones col kept intact)
                    ps = ps_g1.tile([128, 512], f32, tag="g1")
                    for c in range(8):
                        nc.tensor.matmul(
                            ps[:],
                            xT_sb[:, c, tt * 128 : (tt + 1) * 128],
                            wv_sb[:, c, :],
                            start=(c == 0),
                            stop=(c == 7),
                        )
                    nc.vector.tensor_tensor(
                        von[:, tt, :, 0:64],
                        ps[:].rearrange("p (h d) -> p h d", d=64),
                        bvb_sb[:].rearrange("p (h d) -> p h d", d=64),
                        mybir.AluOpType.add,
                    )

                def gemm1a_chunk(fc):
                    # q,k transposed layout [f, t]
                    for tt in range(GQ):
                        ps = ps_g1.tile([128, 512], f32, tag="g1")
                        if g1a_fp8:
                            # DoubleRow: 2 fp8 K-planes per PE cell, K=256/mm
                            for c2 in range(4):
                                nc.tensor.matmul(
                                    ps[:],
                                    wqk8_sb[:, 2 * c2 : 2 * c2 + 2,
                                            fc * 128 : (fc + 1) * 128],
                                    xT8_sb[:, 2 * c2 : 2 * c2 + 2,
                                           tt * 512 : (tt + 1) * 512],
                                    start=(c2 == 0),
                                    stop=(c2 == 3),
                                    perf_mode=mybir.MatmulPerfMode.DoubleRow,
                                )
                        else:
                            for c in range(8):
                                nc.tensor.matmul(
                                    ps[:],
                                    wqk_sb[:, c, fc * 128 : (fc + 1) * 128],
                                    xT_sb[:, c, tt * 512 : (tt + 1) * 512],
                                    start=(c == 0),
                                    stop=(c == 7),
                                )
                        nc.vector.tensor_scalar(
                            qkT[:, fc, tt * 512 : (tt + 1) * 512],
                            ps[:],
                            bqk_sb[:, fc : fc + 1],
                            None,
                            mybir.AluOpType.add,
                        )

                def flash_pair_qt(i, qt):
                    # heads 2i (partitions 0-63) and 2i+1 (partitions 64-127)
                    nkc = 4 * (qt + 1)
                    a_ps = [
                        ps_a.tile([65, 512], f32, tag="aT", name=f"aps{i}_{qt}_{h}")
                        for h in range(2)
                    ]

                    def scores(kc):
                        # both heads' QK^T into one 2-bank psum tile, one
                        # fused exp, one fused causal mask. Diagonal chunks
                        # skip their fully-masked leading columns entirely
                        # (q < k for all k in the chunk): width 512-128j.
                        j = kc - 4 * qt  # >=0 -> diagonal-overlap chunk
                        off = 128 * j if j > 0 else 0
                        sT = ps_s.tile(
                            [128, 2, 512], f32, tag="sT", name=f"sT{i}_{qt}_{kc}"
                        )
                        for h in range(2):
                            p0 = 64 * h
                            nc.tensor.matmul(
                                sT[:, h, off:],
                                qkT[p0 : p0 + 64, 4 + i, kc * 128 : (kc + 1) * 128],
                                qkT[p0 : p0 + 64, i,
                                    qt * 512 + off : (qt + 1) * 512],
                                start=True,
                                stop=True,
                            )
                        pT = ptmp.tile(
                            [128, 2, 512], bf16, tag="pT", name=f"pT{i}_{qt}_{kc}"
                        )
                        nc.scalar.activation(
                            pT[:, :, off:],
                            sT[:, :, off:],
                            mybir.ActivationFunctionType.Exp,
                            bias=zb[:],
                            scale=0.125,
                        )
                        if j >= 0:
                            nc.vector.tensor_tensor(
                                pT[:, :, off:],
                                pT[:, :, off:],
                                msk_sb[:, j : j + 1, off:].to_broadcast(
                                    (128, 2, 512 - off)
                                ),
                                mybir.AluOpType.mult,
                            )
                        return pT, off

                    def pv(kc, pT, off):
                        for h in range(2):
                            nc.tensor.matmul(
                                a_ps[h][:, off:],
                                von[:, kc, 2 * i + h, :],
                                pT[:, h, off:],
                                start=(kc == 0),
                                stop=(kc == nkc - 1),
                            )

                    # software pipeline: PV lags scores by one chunk so the
                    # PE never stalls on the ACT exp of the current chunk
                    prev = scores(0)
                    for kc in range(1, nkc):
                        cur = scores(kc)
                        pv(kc - 1, *prev)
                        prev = cur
                    pv(nkc - 1, *prev)

                    # normalize by the ones-row denominator and store bf16
                    for h in range(2):
                        rec = ptmp2.tile([1, 512], f32, tag="rec")
                        nc.vector.reciprocal(rec[:], a_ps[h][64:65, :])
                        rb_sb = ptmp2.tile([64, 512], f32, tag="rbs")
                        if bc_mode == "gpsimd":
                            nc.gpsimd.partition_broadcast(rb_sb[:], rec[:])
                        else:
                            rb = ps_r.tile([64, 512], f32, tag="rb")
                            nc.tensor.matmul(rb[:], ones1[:], rec[:],
                                             start=True, stop=True)
                            nc.vector.tensor_copy(rb_sb[:], rb[:])
                        nc.vector.tensor_tensor(
                            aT[64 * h : 64 * h + 64, i, qt * 512 : (qt + 1) * 512],
                            a_ps[h][0:64, :],
                            rb_sb[:],
                            mybir.AluOpType.mult,
                        )

                def allgather_pair(i):
                    # ship pair i's attention output while later pairs compute
                    nc.sync.dma_start(ag_ins[i][:], aT[:, i, :])
                    if sim_single or no_cc:
                        nc.sync.dma_start(ag_outs[i][0:128, :], ag_ins[i][:])
                        nc.sync.dma_start(ag_outs[i][128:256, :], ag_ins[i][:])
                    else:
                        nc.gpsimd.collective_compute(
                            "AllGather",
                            mybir.AluOpType.bypass,
                            replica_groups=groups,
                            ins=[ag_ins[i][:].opt()],
                            outs=[ag_outs[i][:].opt()],
                        )
                    # prefetch into SBUF for GEMM2 while flash continues
                    for g in range(2):
                        nc.sync.dma_start(
                            gath[:, g * 4 + i, :],
                            ag_outs[i][g * 128 : (g + 1) * 128, :],
                        )

                def gemm2_pass1():
                    # first half of c_proj contraction (pairs 0,1 of both
                    # groups, already AllGathered) hidden under flash pairs
                    # 2-3; writes fp32 partials to `out`, pass2 accumulates.
                    for tt in range(TT16):
                        ps = ps_g1.tile([128, 512], f32, tag="g1")
                        for ci, c in enumerate([0, 4, 1, 5]):
                            nc.tensor.matmul(
                                ps[:],
                                gath[:, c, tt * 128 : (tt + 1) * 128],
                                wpj_sb[:, c, :],
                                start=(ci == 0),
                                stop=(ci == 3),
                            )
                        og = oev.tile([128, 512], f32, tag="og")
                        nc.vector.tensor_tensor(
                            og[:], ps[:], bpj_sb[:], mybir.AluOpType.add
                        )
                        nc.sync.dma_start(out[tt * 128 : (tt + 1) * 128, :], og[:])

                # GEMM1a for pair 0 first; GEMM1b chunks just-in-time under
                # flash pair 0; later pairs' GEMM1a between flash pairs.
                gemm1a_chunk(0)
                gemm1a_chunk(4)
                for i in range(4):
                    if phase_limit >= 2:
                        for qt in range(GQ):
                            if i == 0:
                                for tt in range(4 * qt, 4 * qt + 4):
                                    gemm1b_chunk(tt)
                            flash_pair_qt(i, qt)
                    elif i == 0:
                        for tt in range(TT16):
                            gemm1b_chunk(tt)
                    if i < 3:
                        gemm1a_chunk(i + 1)
                        gemm1a_chunk(5 + i)
                    if phase_limit >= 3:
                        allgather_pair(i)
                        if i == 2:
                            gemm2_pass1()

            if phase_limit < 3:
                continue
            # ---- GEMM2 pass2: remaining contraction chunks (pairs 2,3),
            # accumulated into `out` by the DMA engine's inline adder.
            with tc.tile_pool(name="ps_g2", bufs=2, space="PSUM") as ps_g2:
                for tt in range(TT16):
                    ps = ps_g2.tile([128, 512], f32, tag="g2")
                    for ci, c in enumerate([2, 6, 3, 7]):
                        nc.tensor.matmul(
                            ps[:],
                            gath[:, c, tt * 128 : (tt + 1) * 128],
                            wpj_sb[:, c, :],
                            start=(ci == 0),
                            stop=(ci == 3),
                        )
                    og = oev.tile([128, 512], f32, tag="og")
                    nc.vector.tensor_copy(og[:], ps[:])
                    nc.gpsimd.dma_start(
                        out[tt * 128 : (tt + 1) * 128, :], og[:],
                        accum_op=mybir.AluOpType.add,
                    )

            if debug_taps:
                tq = oev.tile([128, 8, S], f32, tag="tapq")
                nc.vector.tensor_copy(tq[:], qkT[:])
                nc.sync.dma_start(qkT_tap[:], tq[:])
                tv = oev.tile([128, TT16, HPC, 65], f32, tag="tapv")
                nc.vector.tensor_copy(tv[:], von[:])
                nc.sync.dma_start(von_tap[:], tv[:])
                ta = oev.tile([128, 4, S], f32, tag="tapa")
                nc.vector.tensor_copy(ta[:], aT[:])
                nc.sync.dma_start(aT_tap[:], ta[:])

    nc.compile()
    _BUILD_CACHE[key] = nc
    return nc


def make_in_maps(x, w_attn, b_attn, w_proj, b_proj):
    """Shard the full inputs into 8 per-core input maps."""
    x = np.asarray(x, dtype=np.float32)
    w_attn = np.asarray(w_attn, dtype=np.float32)
    b_attn = np.asarray(b_attn, dtype=np.float32)
    w_proj = np.asarray(w_proj, dtype=np.float32)
    b_proj = np.asarray(b_proj, dtype=np.float32)

    kp = np.arange(128)[:, None, None]
    jj = np.arange(4)[None, :, None]
    qf = np.arange(512)[None, None, :]
    mask = (kp + 128 * jj <= qf).astype(BF16)

    in_maps = []
    for c in range(N_CORES):
        b, g = c // 2, c % 2
        sl = slice(g * FQK, (g + 1) * FQK)
        wq = w_attn[:, 0 * NX :][:, sl]
        wk = w_attn[:, 1 * NX :][:, sl]
        wv_ = w_attn[:, 2 * NX :][:, sl]
        bq = b_attn[0 * NX :][sl]
        bk = b_attn[1 * NX :][sl]
        bv_ = b_attn[2 * NX :][sl]
        xt = np.ascontiguousarray(x[b].T)
        wqk_full = np.ascontiguousarray(np.concatenate([wq, wk], axis=1))
        in_maps.append(
            {
                "xT": xt.astype(BF16),
                "xT8": xt.astype(F8NP),
                "wqk": wqk_full.astype(BF16),
                "wqk8": wqk_full.astype(F8NP),
                "wv": np.ascontiguousarray(wv_).astype(BF16),
                "bqk": np.ascontiguousarray(
                    np.concatenate([bq, bk]).reshape(8, 128).T
                ).astype(np.float32),
                "bvb": np.ascontiguousarray(
                    np.broadcast_to(bv_[None, :], (128, FQK))
                ).astype(np.float32),
                "wpj": np.ascontiguousarray(w_proj[:, sl]).astype(BF16),
                "bpj": np.ascontiguousarray(
                    np.broadcast_to(b_proj[None, sl], (128, FQK))
                ).astype(np.float32),
                "msk": mask,
            }
        )
    return in_maps


def assemble_out(results):
    out = np.empty((B, S, NX), dtype=np.float32)
    for c in range(N_CORES):
        b, g = c // 2, c % 2
        out[b, :, g * FQK : (g + 1) * FQK] = results[c]["out"]
    return out


def kernel(x, w_attn, b_attn, w_proj, b_proj):
    nc = build_nc()
    in_maps = make_in_maps(x, w_attn, b_attn, w_proj, b_proj)
    res = run_bass_kernel_spmd(nc, in_maps, core_ids=list(range(N_CORES)))
    return assemble_out(res.results)
